# revision 23
# baseline (speedup 1.0000x reference)
"""Trainium2 Bass kernel for a 12-head causal attention block (GPT-2 style).

Problem: x:[4,2048,768] -> qkv = x@W_attn+b_attn, causal softmax attention
(12 heads, d=64), out @ W_proj + b_proj.

Sharding over 8 NeuronCores: core c handles batch b=c//2 (data parallel) and
head-group hg=c%2 (6 heads = 3 head-pairs, tensor parallel on the qkv
columns / proj rows).  Each core returns a partial projection output; the
host sums the two head-group partials per batch and adds b_proj.

v3 design (from the 226us v2 baseline; trace-driven changes):
  - lead-in: only the first-group-critical w columns (m=0 qT, m=3 kT,
    pair-0 v) are loaded up front, as ONE strided DMA per k-chunk; the
    complement streams in later via the deadline queue.  First matmul
    starts ~4us instead of ~14us.
  - normalize chain: single [65,1024] avsb staging tile for both heads so
    the denominator reshape and the reciprocal redistribute are ONE DMA
    each (was 2+2); those DMAs issue from the gpsimd queue (650ns, idle)
    instead of sync, which the v2 trace showed at >100% occupancy.
  - pair-2 normalize/proj deadlines tightened so each group's proj tiles
    emit during the NEXT group's j-loop; v2 left ~2 groups' worth of proj
    + normalize draining after the last exp (36us tail, now ~12us).
  - y stores batched 2 seq-tiles per DMA (8 issues instead of 16).
  - ones/bias_v DMAs skipped when the v bias is zero.
"""

import os
import ml_dtypes
import numpy as np

N_HEAD = 12
N_EMBD = 768
HEAD_DIM = 64
B, S = 4, 2048
N_CORES = 8
HG_HEADS = 6            # heads per core (3 pairs)
HG_DIM = HG_HEADS * HEAD_DIM   # 384
QKV_W = 3 * HG_DIM      # 1152 qkv columns per core
N_PAIRS = 3
ST = S // 128           # 16 seq tiles of 128
NG = S // 512           # 4 seq groups of 512

LAST_RESULTS = None
_PROGRAMS = {}


def _build_program(skip_vbias=False):
    import concourse.bacc as bacc
    import concourse.tile as tile
    from concourse import mybir

    F32 = mybir.dt.float32
    BF16 = mybir.dt.bfloat16
    AF = mybir.ActivationFunctionType

    nc = bacc.Bacc(None, target_bir_lowering=False)
    # host-packed xT, [128, 12288]: col g*3072 + k*512 + s holds
    # xT[k*128+p, g*512+s] -- each 512-seq quarter is one contiguous 2D DMA
    # (the naive [768,2048] layout needed a 768-descriptor strided transfer
    # that sat on the critical path for ~7us).
    xT_d = nc.declare_dram_parameter("xT", [128, 4 * 3072], BF16, isOutput=False)
    # host-packed qkv weights, [128, 6912]: cols 0:2304 hold the
    # first-attention-group-critical blocks {m0-qT, m3-kT, pair0-v} k-major
    # (384 per k-chunk), cols 2304:6912 the complement {m1, m2, m4, m5,
    # v1, v2} k-major (768 per k-chunk) -- so the critical lead-in load and
    # the deferred load are ONE contiguous 2D DMA each.
    wqkv_d = nc.declare_dram_parameter("w_qkv", [128, 54 * 128], BF16, isOutput=False)
    bqk_d = nc.declare_dram_parameter("b_qk", [768], F32, isOutput=False)
    bv_d = nc.declare_dram_parameter("b_v", [HG_DIM], BF16, isOutput=False)
    wproj_d = nc.declare_dram_parameter("w_proj", [HG_DIM, N_EMBD], BF16, isOutput=False)
    ones_d = nc.declare_dram_parameter("ones", [1, 128], BF16, isOutput=False)
    # y partials in bf16: halves the store traffic (the host sums the two
    # head-group partials in fp32; bf16 partial rounding adds ~0.1% error)
    y_d = nc.declare_dram_parameter("y", [S, N_EMBD], BF16, isOutput=True)

    with tile.TileContext(nc) as tc:
        from contextlib import ExitStack

        with ExitStack() as outer:
            consts = outer.enter_context(tc.tile_pool(name="consts", bufs=1))
            ones_row = consts.tile([1, 128], BF16)
            bias_v = consts.tile([1, HG_DIM], BF16)
            if not skip_vbias:
                nc.gpsimd.dma_start(out=ones_row[:], in_=ones_d[:])
                nc.gpsimd.dma_start(
                    out=bias_v[:], in_=bv_d[0:HG_DIM].rearrange("(o v) -> o v", o=1)
                )
            bias_qk = consts.tile([128, 6], F32)      # col m: b_qk[128m:128m+128]
            nc.gpsimd.dma_start(
                out=bias_qk[:], in_=bqk_d[0:768].rearrange("(m p) -> p m", p=128)
            )

            # ---- persistent activations/weights in SBUF (all bf16) ----
            big = outer.enter_context(tc.tile_pool(name="big", bufs=1))
            xT = big.tile([128, 6 * S], BF16)       # [emb-part, k-chunk*2048+seq]
            w_all = big.tile([128, 54 * 128], BF16)  # packed layout (see wqkv_d)

            def wcol(k, which):
                # column of 128-wide weight block `which` of k-chunk k in the
                # packed w_all layout
                ci = {"m0": 0, "m3": 1, "v0": 2}
                if which in ci:
                    return k * 384 + ci[which] * 128
                ri = {"m1": 0, "m2": 1, "m4": 2, "m5": 3, "v1": 4, "v2": 5}
                return 2304 + k * 768 + ri[which] * 128
            w_proj = big.tile([128, N_PAIRS * N_EMBD], BF16)
            qkT = big.tile([128, 6 * S], BF16)      # m=0..2 qT pairs, m=3..5 kT pairs
            # per k-tile: 6 heads x (64 v-cols + a ones col for the softmax
            # denominator) -> P@V and row-sums come from one M=65 matmul
            v_all = big.tile([128, ST * 390], BF16)  # [seq, t*390 + 65h + d]
            attnT = big.tile([128, N_PAIRS * S], BF16)

            nc.gpsimd.memset(v_all[:], 1.0)
            # CRITICAL lead-in inputs as SINGLE multi-dim strided DMAs (each
            # dma_start costs ~0.6us of ISSUE time on its trigger engine, so
            # issue count is what matters): the w columns the first attention
            # group needs ({0:128 m0-qT, 384:512 m3-kT, 768:896 pair0-v} per
            # k-chunk) in one DMA on sync, and the xT g0 quarter (cols 0:512
            # of every k-chunk) in one DMA on scalar, in parallel.
            nc.sync.dma_start(out=w_all[:, 0:2304], in_=wqkv_d[:, 0:2304])
            xT_view_s = xT[:].rearrange("p (k s) -> p k s", k=6)
            nc.scalar.dma_start(out=xT_view_s[:, :, 0:512],
                                in_=xT_d[:, 0:3072])

            # deferred inputs (one contiguous DMA each), deadline-queued on
            # sync behind the critical lead-in transfers.
            def emit_w_rest():
                nc.sync.dma_start(out=w_all[:, 2304:6912], in_=wqkv_d[:, 2304:6912])

            def emit_xT_quarter(g):
                nc.sync.dma_start(out=xT_view_s[:, :, g * 512:(g + 1) * 512],
                                  in_=xT_d[:, g * 3072:(g + 1) * 3072])

            def emit_wproj():
                nc.sync.dma_start(
                    out=w_proj[:].rearrange("p (c e) -> p c e", c=3),
                    in_=wproj_d[:].rearrange("(c p) e -> p c e", p=128),
                )

            # ---- pools ----
            stps = outer.enter_context(tc.tile_pool(name="stps", bufs=2, space="PSUM"))
            avps = outer.enter_context(tc.tile_pool(name="avps", bufs=2, space="PSUM"))
            auxps = outer.enter_context(tc.tile_pool(name="auxps", bufs=2, space="PSUM"))
            ptp = outer.enter_context(tc.tile_pool(name="ptp", bufs=4))
            avsb = outer.enter_context(tc.tile_pool(name="avsb", bufs=3))
            rcp = outer.enter_context(tc.tile_pool(name="rcp", bufs=4))
            bcp = outer.enter_context(tc.tile_pool(name="bcp", bufs=4))
            shtmp = outer.enter_context(tc.tile_pool(name="shtmp", bufs=2))
            ystage = outer.enter_context(tc.tile_pool(name="ystage", bufs=2))

            v_view = v_all[:].rearrange("p (t h c) -> p t h c", t=ST, h=HG_HEADS)

            # ---- work-unit emitters (each emits a small PE-dense chunk) ----
            def emit_qk_group(m, g):
                # qkT[:, m*S + g*512 : +512] = (W[:, m-block].T @ xT)[:, g-block] + bias
                ps = auxps.tile([128, 512], F32, tag="aux")
                for k in range(6):
                    wc = wcol(k, f"m{m}")
                    nc.tensor.matmul(
                        ps[:],
                        w_all[:, wc:wc + 128],
                        xT[:, k * S + g * 512:k * S + (g + 1) * 512],
                        start=(k == 0), stop=(k == 5),
                    )
                nc.vector.tensor_scalar_add(
                    qkT[:, m * S + g * 512:m * S + (g + 1) * 512],
                    ps[:], bias_qk[:, m:m + 1],
                )

            def emit_v_tile(pair, t):
                # v rows t*128.. for this pair's two heads (N=128); split by
                # pair so each attention slot computes only its own v work
                ps = auxps.tile([128, 128], F32, tag="aux")
                for k in range(6):
                    wc = wcol(k, f"v{pair}")
                    nc.tensor.matmul(
                        ps[:],
                        xT[:, k * S + t * 128:k * S + (t + 1) * 128],
                        w_all[:, wc:wc + 128],
                        start=(k == 0), stop=(skip_vbias and k == 5),
                    )
                if not skip_vbias:
                    nc.tensor.matmul(   # += ones^T[1,128].T @ bias_v[1,128]
                        ps[:], ones_row[:],
                        bias_v[:, pair * 128:(pair + 1) * 128],
                        start=False, stop=True,
                    )
                nc.vector.tensor_copy(
                    v_view[:, t, 2 * pair:2 * pair + 2, 0:64],
                    ps[:].rearrange("p (h d) -> p h d", h=2),
                )

            ys_pending = {}

            def emit_proj_tile(t, drain=False):
                # stage into the left/right half of a 2-tile ystage buffer;
                # the odd tile of each pair issues one batched y DMA.  In the
                # post-exp drain the PSUM->stage copies run on the (now idle)
                # ACT engine so they never queue behind DVE normalize work.
                psA = auxps.tile([128, 512], F32, tag="aux")
                psB = auxps.tile([128, 256], F32, tag="aux")
                for p in range(N_PAIRS):
                    lhsT = attnT[:, p * S + t * 128:p * S + (t + 1) * 128]
                    nc.tensor.matmul(psA[:], lhsT, w_proj[:, p * N_EMBD:p * N_EMBD + 512],
                                     start=(p == 0), stop=(p == N_PAIRS - 1))
                    nc.tensor.matmul(psB[:], lhsT,
                                     w_proj[:, p * N_EMBD + 512:(p + 1) * N_EMBD],
                                     start=(p == 0), stop=(p == N_PAIRS - 1))
                if t % 2 == 0:
                    ys = ystage.tile([128, 2 * N_EMBD], BF16, tag="ys")
                    ys_pending[t] = ys
                else:
                    ys = ys_pending.pop(t - 1)
                half = (t % 2) * N_EMBD
                if drain:
                    AFc = mybir.ActivationFunctionType.Copy
                    nc.scalar.activation(ys[:, half:half + 512], psA[:], AFc)
                    nc.scalar.activation(ys[:, half + 512:half + 768], psB[:], AFc)
                else:
                    nc.vector.tensor_copy(ys[:, half:half + 512], psA[:])
                    nc.vector.tensor_copy(ys[:, half + 512:half + 768], psB[:])
                if t % 2 == 1:
                    b = t // 2
                    nc.sync.dma_start(
                        out=y_d[b * 256:(b + 1) * 256, :]
                            .rearrange("(i p) e -> p i e", p=128),
                        in_=ys[:].rearrange("p (i e) -> p i e", i=2),
                    )

            # ---- deadline-driven background work queue ----
            # Attention groups execute in a fixed order; (pair, g, j) maps to
            # a global step.  Each qkv/proj work unit carries the step by
            # which it MUST be emitted (Tile deps are emission-order-based:
            # a read emitted before its producer gets no dependency).  Units
            # are pulled with LOOKAHEAD steps of slack so the PE always has
            # background matmuls to chew on while ACT runs exp.
            # pair-2 groups run [1,0,3,2]: each group's normalize + proj
            # tiles emit early in the FOLLOWING group (tight deadlines), so
            # after the last exp only group g2's normalize + proj t8-11
            # remain.
            group_order = {0: [0, 1, 2, 3], 1: [0, 1, 2, 3], 2: [1, 0, 3, 2]}
            step_base = {}
            _acc = 0
            for _p in range(N_PAIRS):
                for _g in group_order[_p]:
                    step_base[(_p, _g)] = _acc
                    _acc += 4 * _g + 4
            TOTAL_STEPS = _acc
            LOOKAHEAD = 9

            work_q = []   # sorted list of (deadline_step, seq, fn)
            _seq = [0]

            def push(deadline, fn):
                import bisect
                _seq[0] += 1
                bisect.insort(work_q, (deadline, _seq[0], fn))

            def pull_work(cur_step):
                # overdue units MUST emit now (correctness: emission order
                # defines Tile dependencies); otherwise spread at one unit
                # per step so the background work stays evenly interleaved.
                while work_q and work_q[0][0] <= cur_step:
                    work_q.pop(0)[2]()
                if work_q and work_q[0][0] <= cur_step + LOOKAHEAD:
                    work_q.pop(0)[2]()

            # ---- attention group with interleaved background units ----
            def emit_attn_group(pair, g):
                q0 = pair * S
                k0 = (3 + pair) * S
                njt = 4 * g + 4
                av0 = avps.tile([65, 512], F32, tag="av")
                av1 = avps.tile([65, 512], F32, tag="av")
                sts = {}
                pts = {}

                def scores(j):
                    diag_r = j - 4 * g
                    c0 = 128 * diag_r if diag_r >= 0 else 0
                    st = stps.tile([128, 1024], F32, tag="st")
                    nc.tensor.matmul(
                        st[:, c0:512],
                        qkT[0:64, k0 + j * 128:k0 + (j + 1) * 128],
                        qkT[0:64, q0 + g * 512 + c0:q0 + (g + 1) * 512],
                        start=True, stop=True, tile_position=(0, 0),
                    )
                    nc.tensor.matmul(
                        st[:, 512 + c0:1024],
                        qkT[64:128, k0 + j * 128:k0 + (j + 1) * 128],
                        qkT[64:128, q0 + g * 512 + c0:q0 + (g + 1) * 512],
                        start=True, stop=True, tile_position=(64, 0),
                    )
                    sts[j] = (st, c0)

                def expmask(j):
                    st, c0 = sts.pop(j)
                    pt = ptp.tile([128, 1024], BF16, tag="pt")
                    nc.scalar.activation(pt[:, c0:1024], st[:, c0:1024],
                                         AF.Exp, bias=0.0, scale=0.125)
                    diag_r = j - 4 * g
                    if diag_r >= 0:
                        for h in range(2):
                            nc.gpsimd.affine_select(
                                out=pt[:, h * 512 + c0:h * 512 + c0 + 128],
                                in_=pt[:, h * 512 + c0:h * 512 + c0 + 128],
                                compare_op=mybir.AluOpType.is_ge,
                                fill=0.0, base=0,
                                pattern=[[1, 128]], channel_multiplier=-1,
                            )
                    pts[j] = (pt, c0)

                def av(j):
                    pt, c0 = pts.pop(j)
                    first, last = (j == 0), (j == njt - 1)
                    for h, avt in ((0, av0), (1, av1)):
                        nc.tensor.matmul(
                            avt[0:65, c0:512],
                            v_all[:, j * 390 + (2 * pair + h) * 65:
                                  j * 390 + (2 * pair + h) * 65 + 65],
                            pt[:, h * 512 + c0:(h + 1) * 512],
                            start=first, stop=last,
                        )

                scores(0)
                expmask(0)
                base = step_base[(pair, g)]
                for j in range(njt):
                    if j + 1 < njt:
                        scores(j + 1)
                        expmask(j + 1)
                    pull_work(base + j)
                    av(j)

                # evacuate the AV accumulators to SBUF (one copy per head into
                # a shared [65,1024] staging tile -- frees the PSUM banks for
                # the next group's AV almost immediately); the
                # recip/redistribute/multiply chain is DEFERRED into the next
                # group's instruction stream so it never stalls the PE at the
                # group boundary.
                avs = avsb.tile([65, 1024], F32, tag="avsb")
                nc.vector.tensor_copy(avs[:, 0:512], av0[:])
                nc.vector.tensor_copy(avs[:, 512:1024], av1[:])

                final = (pair == 2 and g == group_order[2][-1])
                # with the consolidated input DMAs the sync queue is near
                # idle mid-kernel, so all normalize DMAs ride it (HWDGE; the
                # gpsimd SWDGE path costs ~1us + library reloads and stalled
                # the chain behind affine_selects in practice).
                dma_eng = nc.sync

                def normalize():
                    cols = slice(pair * S + g * 512, pair * S + (g + 1) * 512)
                    # DVE reciprocal runs ~9 cyc/elem PER LANE: on [1,1024]
                    # it would cost ~6us.  Reshape both heads' denominators
                    # to [128,8] via ONE SBUF DMA (flat row-major pairing:
                    # partition p <- cols 8p..8p+7, so p<64 is head0) so the
                    # recip uses 128 lanes (~0.2us), then shape back to
                    # [2,512] rows for the gpsimd partition broadcasts.
                    dn8 = rcp.tile([128, 8], F32, tag="dn8")
                    dma_eng.dma_start(out=dn8[:], in_=avs[64:65, :])
                    rc8 = rcp.tile([128, 8], F32, tag="rc8")
                    with nc.allow_low_precision(reason="softmax normalize bf16"):
                        nc.vector.reciprocal(rc8[:], dn8[:])
                        rc2 = rcp.tile([1, 1024], F32, tag="rc2")
                        dma_eng.dma_start(out=rc2[:], in_=rc8[:])
                        for h in range(2):
                            bc = bcp.tile([64, 512], F32)
                            nc.gpsimd.partition_broadcast(
                                bc[:], rc2[:, h * 512:(h + 1) * 512], channels=64)
                            if h == 0:
                                nc.vector.tensor_mul(attnT[0:64, cols],
                                                     avs[0:64, 0:512], bc[:])
                            else:
                                # DVE lanes are partition-locked: odd head's
                                # rows 64-127 via an SBUF bounce + DMA shift
                                tmp = shtmp.tile([64, 512], BF16)
                                nc.vector.tensor_mul(tmp[:], avs[0:64, 512:1024],
                                                     bc[:])
                                nc.sync.dma_start(out=attnT[64:128, cols],
                                                  in_=tmp[:])

                nxt = base + njt
                if pair == 2:
                    # tight deadlines: normalize pops at the next group's
                    # step 0 (eligible from nxt+1-LOOKAHEAD, head of queue by
                    # (deadline, seq)), proj tiles follow one per step.  For
                    # the final group nxt == TOTAL_STEPS and these drain
                    # immediately after the j-loop, in push order.
                    push(nxt + 1, normalize)
                    for i, t in enumerate(range(4 * g, 4 * g + 4)):
                        push(nxt + 2 + i, lambda t=t: emit_proj_tile(t, drain=final))
                else:
                    # pairs 0/1: keep the relaxed deadline so the broadcast
                    # queues behind the next group's first affine_selects.
                    push(nxt + LOOKAHEAD, normalize)

            # ================= schedule =================
            # upfront: just enough qkv for attn(0, g0); v t0-3 go through
            # the deadline queue (first read at av(j=t) of group (0,0))
            emit_qk_group(3, 0)          # kT pair 0, seq 0-511
            emit_qk_group(0, 0)          # qT pair 0, seq 0-511

            # deadlines: qT(p, g) is read only by group (p, g); kT(p, g') is
            # read by EVERY group (p, g >= g'), so its deadline is the
            # earliest-executing such group - for pair 2 (non-monotone group
            # order) that is the first group of the pair for ALL kT chunks.
            # qT/kT run THREE steps early: emitted just-in-time (base-1, as
            # in v2) the next group's first scores wait ~2us for the qk
            # chain + bias add, stalling the exp stream at every group
            # boundary.
            for p in range(N_PAIRS):
                for g in range(NG):
                    if (p, g) == (0, 0):
                        continue
                    kt_dl = min(step_base[(p, gg)] for gg in range(g, NG)) - 3
                    push(kt_dl, lambda m=3 + p, g=g: emit_qk_group(m, g))
                    push(step_base[(p, g)] - 3,
                         lambda m=p, g=g: emit_qk_group(m, g))
            # v(pair, t) is first read at av(j=t) of the earliest-executing
            # group g of that pair with 4g+3 >= t
            for p in range(N_PAIRS):
                for t in range(16):
                    dl = min(step_base[(p, g)]
                             for g in group_order[p] if 4 * g + 3 >= t) + t
                    push(dl, lambda p=p, t=t: emit_v_tile(p, t))
            # deferred w complement: needed first by pair-1 qT/kT/v work
            # (earliest deadline around step_base[(1,0)]-3)
            push(step_base[(0, 2)], emit_w_rest)
            # xT quarter g is first read by qk(0, g) units (deadline base-3,
            # so the DMA must be pushed earlier still to stay the producer)
            for g in range(1, NG):
                push(step_base[(0, g)] - 6, lambda g=g: emit_xT_quarter(g))
            # w_proj is first read by proj units in pair 2
            push(step_base[(1, 0)], emit_wproj)

            for pair in range(N_PAIRS):
                for g in group_order[pair]:
                    emit_attn_group(pair, g)

            # drain in deadline order: the final group's normalize precedes
            # its proj tiles (same-ordered deadlines)
            while work_q:
                work_q.pop(0)[2]()

    nc.compile()
    return nc


def _numpy_fallback(x, mask, W_attn, b_attn, W_proj, b_proj):
    qkv = x @ W_attn + b_attn
    q, k, v = np.split(qkv, 3, axis=-1)

    def heads(t):
        return t.reshape(B, S, N_HEAD, HEAD_DIM).transpose(0, 2, 1, 3)

    q, k, v = heads(q), heads(k), heads(v)
    attn = np.einsum("bhqd,bhkd->bhqk", q, k) / np.sqrt(np.float32(HEAD_DIM))
    attn = attn + mask * (-1e9)
    attn = attn - attn.max(axis=-1, keepdims=True)
    attn = np.exp(attn)
    attn = attn / attn.sum(axis=-1, keepdims=True)
    out = np.einsum("bhqk,bhkd->bhqd", attn, v)
    out = out.transpose(0, 2, 1, 3).reshape(B, S, N_EMBD)
    return (out @ W_proj + b_proj).astype(np.float32)


def _pack_w(Wc):
    """[768, 1152] per-core qkv weight -> [128, 6912] packed layout: cols
    0:2304 = k-major {m0, m3, v0} blocks (the first attention group's
    critical columns), cols 2304:6912 = k-major {m1, m2, m4, m5, v1, v2}."""
    crit = np.concatenate(
        [Wc[:, 0:128], Wc[:, 384:512], Wc[:, 768:896]], axis=1)      # [768, 384]
    rest = np.concatenate(
        [Wc[:, 128:384], Wc[:, 512:768], Wc[:, 896:1152]], axis=1)   # [768, 768]
    critP = crit.reshape(6, 128, 384).transpose(1, 0, 2).reshape(128, 2304)
    restP = rest.reshape(6, 128, 768).transpose(1, 0, 2).reshape(128, 4608)
    return np.concatenate([critP, restP], axis=1)


def make_in_maps(x, W_attn, b_attn, W_proj):
    bf16 = ml_dtypes.bfloat16
    in_maps = []
    for c in range(N_CORES):
        b, hg = divmod(c, 2)
        o = HG_DIM * hg
        Wc = np.concatenate(
            [W_attn[:, o:o + HG_DIM],
             W_attn[:, 768 + o:768 + o + HG_DIM],
             W_attn[:, 1536 + o:1536 + o + HG_DIM]], axis=1)
        xTc = x[b].T.astype(bf16)   # [768, 2048]
        xT_packed = (xTc.reshape(6, 128, 4, 512).transpose(1, 2, 0, 3)
                     .reshape(128, 4 * 3072))
        in_maps.append({
            "xT": np.ascontiguousarray(xT_packed),
            "w_qkv": np.ascontiguousarray(_pack_w(Wc).astype(bf16)),
            "b_qk": np.ascontiguousarray(np.concatenate(
                [b_attn[o:o + HG_DIM], b_attn[768 + o:768 + o + HG_DIM]])),
            "b_v": np.ascontiguousarray(b_attn[1536 + o:1536 + o + HG_DIM]).astype(bf16),
            "w_proj": np.ascontiguousarray(W_proj[o:o + HG_DIM, :].astype(bf16)),
            "ones": np.ones((1, 128), dtype=bf16),
        })
    return in_maps


def kernel(x, mask, W_attn, b_attn, W_proj, b_proj):
    global LAST_RESULTS
    x = np.asarray(x, dtype=np.float32)
    mask = np.asarray(mask, dtype=np.float32)
    W_attn = np.asarray(W_attn, dtype=np.float32)
    b_attn = np.asarray(b_attn, dtype=np.float32)
    W_proj = np.asarray(W_proj, dtype=np.float32)
    b_proj = np.asarray(b_proj, dtype=np.float32)

    # the kernel exploits causal structure; verify the mask actually is causal
    causal = 1.0 - np.tril(np.ones((S, S), dtype=np.float32))
    if mask.shape != (1, 1, S, S) or not np.array_equal(mask[0, 0], causal):
        return _numpy_fallback(x, mask, W_attn, b_attn, W_proj, b_proj)

    from concourse.bass_utils import run_bass_kernel_spmd

    skip_vbias = not b_attn[1536:2304].any()   # v-bias exactly zero
    if skip_vbias not in _PROGRAMS:
        _PROGRAMS[skip_vbias] = _build_program(skip_vbias=skip_vbias)

    in_maps = make_in_maps(x, W_attn, b_attn, W_proj)

    trace = bool(int(os.environ.get("ATTN_KERNEL_TRACE", "0")))
    res = run_bass_kernel_spmd(_PROGRAMS[skip_vbias], in_maps,
                               list(range(N_CORES)), trace=trace)
    LAST_RESULTS = res

    y = np.zeros((B, S, N_EMBD), dtype=np.float32)
    for c in range(N_CORES):
        y[c // 2] += res.results[c]["y"].astype(np.float32)
    y += b_proj
    return y


# revision 24
# speedup vs baseline: 1.1948x; 1.1948x over previous
"""Trainium2 Bass kernel for a 12-head causal attention block (GPT-2 style).

Problem: x:[4,2048,768] -> qkv = x@W_attn+b_attn, causal softmax attention
(12 heads, d=64), out @ W_proj + b_proj.

Sharding over 8 NeuronCores: core c handles batch b=c//2 (data parallel) and
head-group hg=c%2 (6 heads = 3 head-pairs, tensor parallel on the qkv
columns / proj rows).  Each core returns a partial projection output; the
host sums the two head-group partials per batch and adds b_proj.

v3 design (from the 226us v2 baseline; trace-driven changes):
  - lead-in: only the first-group-critical w columns (m=0 qT, m=3 kT,
    pair-0 v) are loaded up front, as ONE strided DMA per k-chunk; the
    complement streams in later via the deadline queue.  First matmul
    starts ~4us instead of ~14us.
  - normalize chain: single [65,1024] avsb staging tile for both heads so
    the denominator reshape and the reciprocal redistribute are ONE DMA
    each (was 2+2); those DMAs issue from the gpsimd queue (650ns, idle)
    instead of sync, which the v2 trace showed at >100% occupancy.
  - pair-2 normalize/proj deadlines tightened so each group's proj tiles
    emit during the NEXT group's j-loop; v2 left ~2 groups' worth of proj
    + normalize draining after the last exp (36us tail, now ~12us).
  - y stores batched 2 seq-tiles per DMA (8 issues instead of 16).
  - ones/bias_v DMAs skipped when the v bias is zero.
"""

import os
import ml_dtypes
import numpy as np

N_HEAD = 12
N_EMBD = 768
HEAD_DIM = 64
B, S = 4, 2048
N_CORES = 8
HG_HEADS = 6            # heads per core (3 pairs)
HG_DIM = HG_HEADS * HEAD_DIM   # 384
QKV_W = 3 * HG_DIM      # 1152 qkv columns per core
N_PAIRS = 3
ST = S // 128           # 16 seq tiles of 128
NG = S // 512           # 4 seq groups of 512

LAST_RESULTS = None
_PROGRAMS = {}


def _build_program(skip_vbias=False):
    import concourse.bacc as bacc
    import concourse.tile as tile
    from concourse import mybir

    F32 = mybir.dt.float32
    BF16 = mybir.dt.bfloat16
    AF = mybir.ActivationFunctionType

    nc = bacc.Bacc(None, target_bir_lowering=False)
    # host-packed xT, [128, 12288]: col g*3072 + k*512 + s holds
    # xT[k*128+p, g*512+s] -- each 512-seq quarter is one contiguous 2D DMA
    # (the naive [768,2048] layout needed a 768-descriptor strided transfer
    # that sat on the critical path for ~7us).
    xT_d = nc.declare_dram_parameter("xT", [128, 4 * 3072], BF16, isOutput=False)
    # host-packed qkv weights, [128, 6912]: cols 0:2304 hold the
    # first-attention-group-critical blocks {m0-qT, m3-kT, pair0-v} k-major
    # (384 per k-chunk), cols 2304:6912 the complement {m1, m2, m4, m5,
    # v1, v2} k-major (768 per k-chunk) -- so the critical lead-in load and
    # the deferred load are ONE contiguous 2D DMA each.
    wqkv_d = nc.declare_dram_parameter("w_qkv", [128, 54 * 128], BF16, isOutput=False)
    bqk_d = nc.declare_dram_parameter("b_qk", [768], F32, isOutput=False)
    bv_d = nc.declare_dram_parameter("b_v", [HG_DIM], BF16, isOutput=False)
    wproj_d = nc.declare_dram_parameter("w_proj", [HG_DIM, N_EMBD], BF16, isOutput=False)
    ones_d = nc.declare_dram_parameter("ones", [1, 128], BF16, isOutput=False)
    # y partials in bf16: halves the store traffic (the host sums the two
    # head-group partials in fp32; bf16 partial rounding adds ~0.1% error)
    y_d = nc.declare_dram_parameter("y", [S, N_EMBD], BF16, isOutput=True)

    with tile.TileContext(nc) as tc:
        from contextlib import ExitStack

        with ExitStack() as outer:
            consts = outer.enter_context(tc.tile_pool(name="consts", bufs=1))
            ones_row = consts.tile([1, 128], BF16)
            bias_v = consts.tile([1, HG_DIM], BF16)
            if not skip_vbias:
                nc.gpsimd.dma_start(out=ones_row[:], in_=ones_d[:])
                nc.gpsimd.dma_start(
                    out=bias_v[:], in_=bv_d[0:HG_DIM].rearrange("(o v) -> o v", o=1)
                )
            bias_qk = consts.tile([128, 6], F32)      # col m: b_qk[128m:128m+128]
            nc.gpsimd.dma_start(
                out=bias_qk[:], in_=bqk_d[0:768].rearrange("(m p) -> p m", p=128)
            )

            # ---- persistent activations/weights in SBUF (all bf16) ----
            big = outer.enter_context(tc.tile_pool(name="big", bufs=1))
            xT = big.tile([128, 6 * S], BF16)       # [emb-part, k-chunk*2048+seq]
            w_all = big.tile([128, 54 * 128], BF16)  # packed layout (see wqkv_d)

            def wcol(k, which):
                # column of 128-wide weight block `which` of k-chunk k in the
                # packed w_all layout
                ci = {"m0": 0, "m3": 1, "v0": 2}
                if which in ci:
                    return k * 384 + ci[which] * 128
                ri = {"m1": 0, "m2": 1, "m4": 2, "m5": 3, "v1": 4, "v2": 5}
                return 2304 + k * 768 + ri[which] * 128
            w_proj = big.tile([128, N_PAIRS * N_EMBD], BF16)
            qkT = big.tile([128, 6 * S], BF16)      # m=0..2 qT pairs, m=3..5 kT pairs
            # per k-tile: 6 heads x (64 v-cols + a ones col for the softmax
            # denominator) -> P@V and row-sums come from one M=65 matmul
            v_all = big.tile([128, ST * 390], BF16)  # [seq, t*390 + 65h + d]
            attnT = big.tile([128, N_PAIRS * S], BF16)

            nc.gpsimd.memset(v_all[:], 1.0)
            # CRITICAL lead-in inputs as SINGLE multi-dim strided DMAs (each
            # dma_start costs ~0.6us of ISSUE time on its trigger engine, so
            # issue count is what matters): the w columns the first attention
            # group needs ({0:128 m0-qT, 384:512 m3-kT, 768:896 pair0-v} per
            # k-chunk) in one DMA on sync, and the xT g0 quarter (cols 0:512
            # of every k-chunk) in one DMA on scalar, in parallel.
            nc.sync.dma_start(out=w_all[:, 0:2304], in_=wqkv_d[:, 0:2304])
            xT_view_s = xT[:].rearrange("p (k s) -> p k s", k=6)
            nc.scalar.dma_start(out=xT_view_s[:, :, 0:512],
                                in_=xT_d[:, 0:3072])

            # deferred inputs (one contiguous DMA each), deadline-queued on
            # sync behind the critical lead-in transfers.
            def emit_w_rest():
                nc.sync.dma_start(out=w_all[:, 2304:6912], in_=wqkv_d[:, 2304:6912])

            def emit_xT_quarter(g):
                nc.sync.dma_start(out=xT_view_s[:, :, g * 512:(g + 1) * 512],
                                  in_=xT_d[:, g * 3072:(g + 1) * 3072])

            def emit_wproj():
                nc.sync.dma_start(
                    out=w_proj[:].rearrange("p (c e) -> p c e", c=3),
                    in_=wproj_d[:].rearrange("(c p) e -> p c e", p=128),
                )

            # ---- pools ----
            stps = outer.enter_context(tc.tile_pool(name="stps", bufs=2, space="PSUM"))
            avps = outer.enter_context(tc.tile_pool(name="avps", bufs=2, space="PSUM"))
            auxps = outer.enter_context(tc.tile_pool(name="auxps", bufs=2, space="PSUM"))
            ptp = outer.enter_context(tc.tile_pool(name="ptp", bufs=4))
            avsb = outer.enter_context(tc.tile_pool(name="avsb", bufs=3))
            rcp = outer.enter_context(tc.tile_pool(name="rcp", bufs=4))
            bcp = outer.enter_context(tc.tile_pool(name="bcp", bufs=4))
            shtmp = outer.enter_context(tc.tile_pool(name="shtmp", bufs=2))
            ystage = outer.enter_context(tc.tile_pool(name="ystage", bufs=2))

            v_view = v_all[:].rearrange("p (t h c) -> p t h c", t=ST, h=HG_HEADS)

            # ---- work-unit emitters (each emits a small PE-dense chunk) ----
            def emit_qk_group(m, g):
                # qkT[:, m*S + g*512 : +512] = (W[:, m-block].T @ xT)[:, g-block] + bias
                ps = auxps.tile([128, 512], F32, tag="aux")
                for k in range(6):
                    wc = wcol(k, f"m{m}")
                    nc.tensor.matmul(
                        ps[:],
                        w_all[:, wc:wc + 128],
                        xT[:, k * S + g * 512:k * S + (g + 1) * 512],
                        start=(k == 0), stop=(k == 5),
                    )
                nc.vector.tensor_scalar_add(
                    qkT[:, m * S + g * 512:m * S + (g + 1) * 512],
                    ps[:], bias_qk[:, m:m + 1],
                )

            def emit_v_tile(pair, t):
                # v rows t*128.. for this pair's two heads (N=128); split by
                # pair so each attention slot computes only its own v work
                ps = auxps.tile([128, 128], F32, tag="aux")
                for k in range(6):
                    wc = wcol(k, f"v{pair}")
                    nc.tensor.matmul(
                        ps[:],
                        xT[:, k * S + t * 128:k * S + (t + 1) * 128],
                        w_all[:, wc:wc + 128],
                        start=(k == 0), stop=(skip_vbias and k == 5),
                    )
                if not skip_vbias:
                    nc.tensor.matmul(   # += ones^T[1,128].T @ bias_v[1,128]
                        ps[:], ones_row[:],
                        bias_v[:, pair * 128:(pair + 1) * 128],
                        start=False, stop=True,
                    )
                nc.vector.tensor_copy(
                    v_view[:, t, 2 * pair:2 * pair + 2, 0:64],
                    ps[:].rearrange("p (h d) -> p h d", h=2),
                )

            ys_pending = {}

            def emit_proj_tile(t, drain=False):
                # stage into the left/right half of a 2-tile ystage buffer;
                # the odd tile of each pair issues one batched y DMA.  In the
                # post-exp drain the PSUM->stage copies run on the (now idle)
                # ACT engine so they never queue behind DVE normalize work.
                psA = auxps.tile([128, 512], F32, tag="aux")
                psB = auxps.tile([128, 256], F32, tag="aux")
                for p in range(N_PAIRS):
                    lhsT = attnT[:, p * S + t * 128:p * S + (t + 1) * 128]
                    nc.tensor.matmul(psA[:], lhsT, w_proj[:, p * N_EMBD:p * N_EMBD + 512],
                                     start=(p == 0), stop=(p == N_PAIRS - 1))
                    nc.tensor.matmul(psB[:], lhsT,
                                     w_proj[:, p * N_EMBD + 512:(p + 1) * N_EMBD],
                                     start=(p == 0), stop=(p == N_PAIRS - 1))
                if t % 2 == 0:
                    ys = ystage.tile([128, 2 * N_EMBD], BF16, tag="ys")
                    ys_pending[t] = ys
                else:
                    ys = ys_pending.pop(t - 1)
                half = (t % 2) * N_EMBD
                if drain:
                    AFc = mybir.ActivationFunctionType.Copy
                    nc.scalar.activation(ys[:, half:half + 512], psA[:], AFc)
                    nc.scalar.activation(ys[:, half + 512:half + 768], psB[:], AFc)
                else:
                    nc.vector.tensor_copy(ys[:, half:half + 512], psA[:])
                    nc.vector.tensor_copy(ys[:, half + 512:half + 768], psB[:])
                if t % 2 == 1:
                    b = t // 2
                    nc.sync.dma_start(
                        out=y_d[b * 256:(b + 1) * 256, :]
                            .rearrange("(i p) e -> p i e", p=128),
                        in_=ys[:].rearrange("p (i e) -> p i e", i=2),
                    )

            # ---- deadline-driven background work queue ----
            # Attention groups execute in a fixed order; (pair, g, j) maps to
            # a global step.  Each qkv/proj work unit carries the step by
            # which it MUST be emitted (Tile deps are emission-order-based:
            # a read emitted before its producer gets no dependency).  Units
            # are pulled with LOOKAHEAD steps of slack so the PE always has
            # background matmuls to chew on while ACT runs exp.
            # pair-2 groups run [1,0,3,2]: each group's normalize + proj
            # tiles emit early in the FOLLOWING group (tight deadlines), so
            # after the last exp only group g2's normalize + proj t8-11
            # remain.
            group_order = {0: [0, 1, 2, 3], 1: [0, 1, 2, 3], 2: [1, 0, 3, 2]}
            step_base = {}
            _acc = 0
            for _p in range(N_PAIRS):
                for _g in group_order[_p]:
                    step_base[(_p, _g)] = _acc
                    _acc += 4 * _g + 4
            TOTAL_STEPS = _acc
            LOOKAHEAD = 9

            work_q = []   # sorted list of (deadline_step, seq, fn)
            _seq = [0]

            def push(deadline, fn):
                import bisect
                _seq[0] += 1
                bisect.insort(work_q, (deadline, _seq[0], fn))

            def pull_work(cur_step):
                # overdue units MUST emit now (correctness: emission order
                # defines Tile dependencies); otherwise spread at one unit
                # per step so the background work stays evenly interleaved.
                while work_q and work_q[0][0] <= cur_step:
                    work_q.pop(0)[2]()
                if work_q and work_q[0][0] <= cur_step + LOOKAHEAD:
                    work_q.pop(0)[2]()

            # ---- attention group with interleaved background units ----
            def emit_attn_group(pair, g):
                q0 = pair * S
                k0 = (3 + pair) * S
                njt = 4 * g + 4
                av0 = avps.tile([65, 512], F32, tag="av")
                av1 = avps.tile([65, 512], F32, tag="av")
                sts = {}
                pts = {}

                def scores(j):
                    diag_r = j - 4 * g
                    c0 = 128 * diag_r if diag_r >= 0 else 0
                    st = stps.tile([128, 1024], F32, tag="st")
                    nc.tensor.matmul(
                        st[:, c0:512],
                        qkT[0:64, k0 + j * 128:k0 + (j + 1) * 128],
                        qkT[0:64, q0 + g * 512 + c0:q0 + (g + 1) * 512],
                        start=True, stop=True, tile_position=(0, 0),
                    )
                    nc.tensor.matmul(
                        st[:, 512 + c0:1024],
                        qkT[64:128, k0 + j * 128:k0 + (j + 1) * 128],
                        qkT[64:128, q0 + g * 512 + c0:q0 + (g + 1) * 512],
                        start=True, stop=True, tile_position=(64, 0),
                    )
                    sts[j] = (st, c0)

                def expmask(j):
                    st, c0 = sts.pop(j)
                    pt = ptp.tile([128, 1024], BF16, tag="pt")
                    nc.scalar.activation(pt[:, c0:1024], st[:, c0:1024],
                                         AF.Exp, bias=0.0, scale=0.125)
                    diag_r = j - 4 * g
                    if diag_r >= 0:
                        for h in range(2):
                            nc.gpsimd.affine_select(
                                out=pt[:, h * 512 + c0:h * 512 + c0 + 128],
                                in_=pt[:, h * 512 + c0:h * 512 + c0 + 128],
                                compare_op=mybir.AluOpType.is_ge,
                                fill=0.0, base=0,
                                pattern=[[1, 128]], channel_multiplier=-1,
                            )
                    pts[j] = (pt, c0)

                def av(j):
                    pt, c0 = pts.pop(j)
                    first, last = (j == 0), (j == njt - 1)
                    for h, avt in ((0, av0), (1, av1)):
                        nc.tensor.matmul(
                            avt[0:65, c0:512],
                            v_all[:, j * 390 + (2 * pair + h) * 65:
                                  j * 390 + (2 * pair + h) * 65 + 65],
                            pt[:, h * 512 + c0:(h + 1) * 512],
                            start=first, stop=last,
                        )

                scores(0)
                expmask(0)
                base = step_base[(pair, g)]
                for j in range(njt):
                    if j + 1 < njt:
                        scores(j + 1)
                        expmask(j + 1)
                    pull_work(base + j)
                    av(j)

                # evacuate the AV accumulators to SBUF (one copy per head into
                # a shared [65,1024] staging tile -- frees the PSUM banks for
                # the next group's AV almost immediately); the
                # recip/redistribute/multiply chain is DEFERRED into the next
                # group's instruction stream so it never stalls the PE at the
                # group boundary.
                avs = avsb.tile([65, 1024], F32, tag="avsb")
                nc.vector.tensor_copy(avs[:, 0:512], av0[:])
                nc.vector.tensor_copy(avs[:, 512:1024], av1[:])

                final = (pair == 2 and g == group_order[2][-1])
                # with the consolidated input DMAs the sync queue is near
                # idle mid-kernel, so all normalize DMAs ride it (HWDGE; the
                # gpsimd SWDGE path costs ~1us + library reloads and stalled
                # the chain behind affine_selects in practice).
                dma_eng = nc.sync

                def normalize():
                    cols = slice(pair * S + g * 512, pair * S + (g + 1) * 512)
                    # DVE reciprocal runs ~9 cyc/elem PER LANE: on [1,1024]
                    # it would cost ~6us.  Reshape both heads' denominators
                    # to [128,8] via ONE SBUF DMA (flat row-major pairing:
                    # partition p <- cols 8p..8p+7, so p<64 is head0) so the
                    # recip uses 128 lanes (~0.2us), then shape back to
                    # [2,512] rows for the gpsimd partition broadcasts.
                    dn8 = rcp.tile([128, 8], F32, tag="dn8")
                    dma_eng.dma_start(out=dn8[:], in_=avs[64:65, :])
                    rc8 = rcp.tile([128, 8], F32, tag="rc8")
                    with nc.allow_low_precision(reason="softmax normalize bf16"):
                        nc.vector.reciprocal(rc8[:], dn8[:])
                        rc2 = rcp.tile([1, 1024], F32, tag="rc2")
                        dma_eng.dma_start(out=rc2[:], in_=rc8[:])
                        for h in range(2):
                            bc = bcp.tile([64, 512], F32)
                            nc.gpsimd.partition_broadcast(
                                bc[:], rc2[:, h * 512:(h + 1) * 512], channels=64)
                            if h == 0:
                                nc.vector.tensor_mul(attnT[0:64, cols],
                                                     avs[0:64, 0:512], bc[:])
                            else:
                                # DVE lanes are partition-locked: odd head's
                                # rows 64-127 via an SBUF bounce + DMA shift
                                tmp = shtmp.tile([64, 512], BF16)
                                nc.vector.tensor_mul(tmp[:], avs[0:64, 512:1024],
                                                     bc[:])
                                nc.sync.dma_start(out=attnT[64:128, cols],
                                                  in_=tmp[:])

                nxt = base + njt
                if pair == 2:
                    # tight deadlines: normalize pops at the next group's
                    # step 0 (eligible from nxt+1-LOOKAHEAD, head of queue by
                    # (deadline, seq)), proj tiles follow one per step.  For
                    # the final group nxt == TOTAL_STEPS and these drain
                    # immediately after the j-loop, in push order.
                    push(nxt + 1, normalize)
                    for i, t in enumerate(range(4 * g, 4 * g + 4)):
                        push(nxt + 2 + i, lambda t=t: emit_proj_tile(t, drain=final))
                else:
                    # pairs 0/1: keep the relaxed deadline so the broadcast
                    # queues behind the next group's first affine_selects.
                    push(nxt + LOOKAHEAD, normalize)

            # ================= schedule =================
            # upfront: just enough qkv for attn(0, g0); v t0-3 go through
            # the deadline queue (first read at av(j=t) of group (0,0))
            emit_qk_group(3, 0)          # kT pair 0, seq 0-511
            emit_qk_group(0, 0)          # qT pair 0, seq 0-511

            # deadlines: qT(p, g) is read only by group (p, g); kT(p, g') is
            # read by EVERY group (p, g >= g'), so its deadline is the
            # earliest-executing such group - for pair 2 (non-monotone group
            # order) that is the first group of the pair for ALL kT chunks.
            # qT/kT for pairs 1/2 run THREE steps early: emitted
            # just-in-time (base-1) the next group's first scores wait ~2us
            # for the qk chain + bias add, stalling the exp stream at every
            # group boundary.  Pair 0's stay just-in-time: its xT quarters
            # are still IN FLIGHT, and emitting compute against an un-landed
            # DMA parks a PSUM buffer + the strict-FIFO PE queue on it
            # (measured +40us!).
            for p in range(N_PAIRS):
                for g in range(NG):
                    if (p, g) == (0, 0):
                        continue
                    slack = 3 if p > 0 else 1
                    kt_dl = min(step_base[(p, gg)] for gg in range(g, NG)) - slack
                    push(kt_dl, lambda m=3 + p, g=g: emit_qk_group(m, g))
                    push(step_base[(p, g)] - slack,
                         lambda m=p, g=g: emit_qk_group(m, g))
            # v(pair, t) is first read at av(j=t) of the earliest-executing
            # group g of that pair with 4g+3 >= t
            for p in range(N_PAIRS):
                for t in range(16):
                    dl = min(step_base[(p, g)]
                             for g in group_order[p] if 4 * g + 3 >= t) + t
                    push(dl, lambda p=p, t=t: emit_v_tile(p, t))
            # deferred w complement: needed first by pair-1 qT/kT/v work
            # (earliest deadline around step_base[(1,0)]-3)
            push(step_base[(0, 2)], emit_w_rest)
            # xT quarter g is first read by qk(0, g) units (deadline base-1)
            for g in range(1, NG):
                push(step_base[(0, g)] - 2, lambda g=g: emit_xT_quarter(g))
            # w_proj is first read by proj units in pair 2
            push(step_base[(1, 0)], emit_wproj)

            for pair in range(N_PAIRS):
                for g in group_order[pair]:
                    emit_attn_group(pair, g)

            # drain in deadline order: the final group's normalize precedes
            # its proj tiles (same-ordered deadlines)
            while work_q:
                work_q.pop(0)[2]()

    nc.compile()
    return nc


def _numpy_fallback(x, mask, W_attn, b_attn, W_proj, b_proj):
    qkv = x @ W_attn + b_attn
    q, k, v = np.split(qkv, 3, axis=-1)

    def heads(t):
        return t.reshape(B, S, N_HEAD, HEAD_DIM).transpose(0, 2, 1, 3)

    q, k, v = heads(q), heads(k), heads(v)
    attn = np.einsum("bhqd,bhkd->bhqk", q, k) / np.sqrt(np.float32(HEAD_DIM))
    attn = attn + mask * (-1e9)
    attn = attn - attn.max(axis=-1, keepdims=True)
    attn = np.exp(attn)
    attn = attn / attn.sum(axis=-1, keepdims=True)
    out = np.einsum("bhqk,bhkd->bhqd", attn, v)
    out = out.transpose(0, 2, 1, 3).reshape(B, S, N_EMBD)
    return (out @ W_proj + b_proj).astype(np.float32)


def _pack_w(Wc):
    """[768, 1152] per-core qkv weight -> [128, 6912] packed layout: cols
    0:2304 = k-major {m0, m3, v0} blocks (the first attention group's
    critical columns), cols 2304:6912 = k-major {m1, m2, m4, m5, v1, v2}."""
    crit = np.concatenate(
        [Wc[:, 0:128], Wc[:, 384:512], Wc[:, 768:896]], axis=1)      # [768, 384]
    rest = np.concatenate(
        [Wc[:, 128:384], Wc[:, 512:768], Wc[:, 896:1152]], axis=1)   # [768, 768]
    critP = crit.reshape(6, 128, 384).transpose(1, 0, 2).reshape(128, 2304)
    restP = rest.reshape(6, 128, 768).transpose(1, 0, 2).reshape(128, 4608)
    return np.concatenate([critP, restP], axis=1)


def make_in_maps(x, W_attn, b_attn, W_proj):
    bf16 = ml_dtypes.bfloat16
    in_maps = []
    for c in range(N_CORES):
        b, hg = divmod(c, 2)
        o = HG_DIM * hg
        Wc = np.concatenate(
            [W_attn[:, o:o + HG_DIM],
             W_attn[:, 768 + o:768 + o + HG_DIM],
             W_attn[:, 1536 + o:1536 + o + HG_DIM]], axis=1)
        xTc = x[b].T.astype(bf16)   # [768, 2048]
        xT_packed = (xTc.reshape(6, 128, 4, 512).transpose(1, 2, 0, 3)
                     .reshape(128, 4 * 3072))
        in_maps.append({
            "xT": np.ascontiguousarray(xT_packed),
            "w_qkv": np.ascontiguousarray(_pack_w(Wc).astype(bf16)),
            "b_qk": np.ascontiguousarray(np.concatenate(
                [b_attn[o:o + HG_DIM], b_attn[768 + o:768 + o + HG_DIM]])),
            "b_v": np.ascontiguousarray(b_attn[1536 + o:1536 + o + HG_DIM]).astype(bf16),
            "w_proj": np.ascontiguousarray(W_proj[o:o + HG_DIM, :].astype(bf16)),
            "ones": np.ones((1, 128), dtype=bf16),
        })
    return in_maps


def kernel(x, mask, W_attn, b_attn, W_proj, b_proj):
    global LAST_RESULTS
    x = np.asarray(x, dtype=np.float32)
    mask = np.asarray(mask, dtype=np.float32)
    W_attn = np.asarray(W_attn, dtype=np.float32)
    b_attn = np.asarray(b_attn, dtype=np.float32)
    W_proj = np.asarray(W_proj, dtype=np.float32)
    b_proj = np.asarray(b_proj, dtype=np.float32)

    # the kernel exploits causal structure; verify the mask actually is causal
    causal = 1.0 - np.tril(np.ones((S, S), dtype=np.float32))
    if mask.shape != (1, 1, S, S) or not np.array_equal(mask[0, 0], causal):
        return _numpy_fallback(x, mask, W_attn, b_attn, W_proj, b_proj)

    from concourse.bass_utils import run_bass_kernel_spmd

    skip_vbias = not b_attn[1536:2304].any()   # v-bias exactly zero
    if skip_vbias not in _PROGRAMS:
        _PROGRAMS[skip_vbias] = _build_program(skip_vbias=skip_vbias)

    in_maps = make_in_maps(x, W_attn, b_attn, W_proj)

    trace = bool(int(os.environ.get("ATTN_KERNEL_TRACE", "0")))
    res = run_bass_kernel_spmd(_PROGRAMS[skip_vbias], in_maps,
                               list(range(N_CORES)), trace=trace)
    LAST_RESULTS = res

    y = np.zeros((B, S, N_EMBD), dtype=np.float32)
    for c in range(N_CORES):
        y[c // 2] += res.results[c]["y"].astype(np.float32)
    y += b_proj
    return y
